# revision 2
# baseline (speedup 1.0000x reference)
"""Mixtral MoE layer (8 experts, top-2, H=2048, I=7168, T=8192) on 8 trn2 NeuronCores.

Expert-parallel: core e owns expert e's FFN weights. Router on host (exact
reference op sequence); host gathers tokens per expert, cores run the FFN,
host scatter-adds weighted outputs back.

v4 vs baseline:
 - fp16 operands instead of bf16 (same matmul speed, 8x lower quantization
   error: rel err 0.41% -> ~0.05%), buying error budget for:
 - the last NB8 inter chunks run phase B in fp8e4 with DoubleRow perf mode
   (2 chunks contracted per instruction at ~1.05 cyc/col vs 2 cols for two
   bf16 instructions => ~1.9x on that slice of the work).
   g for those chunks is produced at scale SG (folded into w3 rows on the
   host), quantized to fp8 by the phase-A DVE mul, and matched with host
   prequantized w2*SW fp8 pairs; their contribution is emitted as a separate
   fp32 output (yt8, scaled by SG*SW) that the host divides and adds.

Device-side layout avoids all on-chip transposes:
  phase A:  Gt[i, c] = silu(W1t.T @ Xt) * (W3t.T @ Xt)   (inter on partitions)
  phase B:  Yt[h, c] += W2t.T @ Gt                        (hidden on partitions)
with Xt = X.T etc., all pre-tiled on host for contiguous DMA runs.
"""

import math

import numpy as np
import ml_dtypes

import concourse.bass as bass
import concourse.mybir as mybir
import concourse.tile as tile
from concourse.bass_utils import run_bass_kernel_spmd

H = 2048          # hidden dim
I = 7168          # intermediate dim
E = 8             # experts = cores
TOPK = 2
HJ = H // 128     # 16 hidden chunks of 128
IGW = 256         # phase-A inter group width
IG = I // IGW     # 28 groups
IK = I // 128     # 56 inter chunks of 128 for phase B
NB8 = 14          # inter chunks (tail) whose phase B runs in fp8 DoubleRow
NP8 = NB8 // 2    # fp8 chunk pairs
IK16 = IK - NB8   # fp16 phase-B chunks
IKG = 7           # fp16 phase-B psum accumulation group size
SG = 8.0          # g scale for fp8 chunks (folded into w3 rows on host);
                  # max |g| measured 14.0 -> g*SG max 112 << 240 (e4m3 max)
SW = 512.0        # w2 scale for fp8 chunks
TBMAX = 512       # token block (matmul free dim)

F16 = mybir.dt.float16
F8 = mybir.dt.float8e4
F32 = mybir.dt.float32

last_exec_time_ns = None  # set when BASS_MOE_TRACE=1
last_results = None


def _install_axon_hooks_shim():
    """This image lacks antenv.axon_hooks (needed by run_bass_kernel_spmd
    trace=True). Provide it, with the NTFF profile hook driven via ctypes
    into the injected axon .so (mirrors trn_agent_boot._ntff_profile_via_ctypes)."""
    import sys

    try:
        import antenv.axon_hooks  # noqa: F401

        return
    except ImportError:
        pass
    import contextlib
    import ctypes
    import types

    hook = None
    so_path = "/opt/axon/libaxon_pjrt.so"
    try:
        lib = ctypes.CDLL(so_path)
        if hasattr(lib, "axon_start_nrt_profile"):
            lib.axon_start_nrt_profile.argtypes = [
                ctypes.POINTER(ctypes.c_int64),
                ctypes.c_size_t,
            ]
            lib.axon_start_nrt_profile.restype = ctypes.c_int64
            lib.axon_stop_nrt_profile.argtypes = [ctypes.c_char_p]
            lib.axon_stop_nrt_profile.restype = ctypes.c_int64

            @contextlib.contextmanager
            def _hook(output_dir, device_ids):
                import jax

                jax.devices()
                if device_ids:
                    ids = (ctypes.c_int64 * len(device_ids))(*device_ids)
                    rc = lib.axon_start_nrt_profile(ids, len(device_ids))
                else:
                    rc = lib.axon_start_nrt_profile(None, 0)
                if rc != 0:
                    raise RuntimeError(f"axon_start_nrt_profile rc={rc}")
                try:
                    yield
                finally:
                    n = lib.axon_stop_nrt_profile(str(output_dir).encode())
                    print(f"ntff profile: {n} file(s) -> {output_dir}", flush=True)

            hook = _hook
    except OSError:
        pass

    mod = types.ModuleType("antenv.axon_hooks")
    mod._hook = hook
    mod.get_axon_ntff_profile_hook = lambda: mod._hook
    mod.set_axon_ntff_profile_hook = lambda h: setattr(mod, "_hook", h)
    sys.modules["antenv.axon_hooks"] = mod


_install_axon_hooks_shim()


def legalize_single_wait(nc):
    """This walrus rejects >1 sem wait per instruction: hoist extras onto
    preceding NoOps on the same engine (per-engine program order preserved)."""
    n_split = 0
    for fn in nc.m.functions:
        for blk in fn.blocks:
            new = []
            for inst in blk.instructions:
                si = inst.sync_info
                if si is not None and si.on_wait and len(si.on_wait) > 1:
                    waits = list(si.on_wait)
                    for i, w in enumerate(waits[:-1]):
                        nop = mybir.InstNoOp(name=f"{inst.name}-w{i}", ins=[], outs=[])
                        nop.engine = inst.engine
                        nop.sync_info = mybir.SyncInfo(on_wait=[w], on_update=[])
                        new.append(nop)
                        n_split += 1
                    inst.sync_info = mybir.SyncInfo(
                        on_wait=[waits[-1]], on_update=list(si.on_update)
                    )
                new.append(inst)
            blk.instructions[:] = new
    return n_split


_programs = {}


def _build_program(C):
    """One SPMD program: FFN for C (padded) tokens of one expert."""
    if C in _programs:
        return _programs[C]

    nc = bass.Bass("TRN2", target_bir_lowering=False, debug=False, num_devices=E)
    xt = nc.declare_dram_parameter("xt", [HJ, 128, C], F16, isOutput=False)
    w1 = nc.declare_dram_parameter("w1", [IK, HJ, 128, 128], F16, isOutput=False)
    w3 = nc.declare_dram_parameter("w3", [IK, HJ, 128, 128], F16, isOutput=False)
    w2 = nc.declare_dram_parameter("w2", [IK16, 128, H], F16, isOutput=False)
    w28 = nc.declare_dram_parameter("w28", [HJ, NP8, 128, 256], F8, isOutput=False)
    yt = nc.declare_dram_parameter("yt", [HJ, 128, C], F16, isOutput=True)
    yt8 = nc.declare_dram_parameter("yt8", [HJ, 128, C], F16, isOutput=True)

    assert C % 16 == 0
    n_blocks = math.ceil(C / TBMAX)
    # near-equal block sizes (all 16-aligned) keep every matmul stream-bound
    base = (C // n_blocks) // 16 * 16
    rem = (C - base * n_blocks) // 16
    tbs = [base + (16 if i < rem else 0) for i in range(n_blocks)]
    offs = [sum(tbs[:i]) for i in range(n_blocks)]
    assert sum(tbs) == C and all(t <= TBMAX for t in tbs)

    n_g16 = math.ceil(IK16 / IKG)  # fp16 phase-B groups (last may be ragged)

    with tile.TileContext(nc) as tc:
        with (
            tc.tile_pool(name="xap", bufs=2) as xap,
            tc.tile_pool(name="xbp", bufs=2) as xbp,
            tc.tile_pool(name="w1p", bufs=3) as w1p,
            tc.tile_pool(name="w3p", bufs=3) as w3p,
            tc.tile_pool(name="w2p", bufs=IKG + 1) as w2p,
            tc.tile_pool(name="w28p", bufs=NP8) as w28p,
            tc.tile_pool(name="gtp", bufs=IK16) as gtp,
            tc.tile_pool(name="gt8p", bufs=NP8) as gt8p,
            tc.tile_pool(name="sip", bufs=2) as sip,
            tc.tile_pool(name="otp", bufs=HJ) as otp,
            tc.tile_pool(name="st8", bufs=2) as st8,
            tc.tile_pool(name="pga", bufs=2, space="PSUM") as pga,
            tc.tile_pool(name="pob", bufs=2, space="PSUM") as pob,
            tc.tile_pool(name="po8", bufs=2, space="PSUM") as po8p,
        ):
            for cb in range(n_blocks):
                c0 = offs[cb]
                tb = tbs[cb]

                xa = xap.tile([128, 4, tb], F16, tag="xa")
                nc.sync.dma_start(
                    out=xa[:, :, :],
                    in_=xt[:4, :, c0 : c0 + tb].rearrange("j p c -> p j c"),
                )
                xb = xbp.tile([128, HJ - 4, tb], F16, tag="xb")
                nc.sync.dma_start(
                    out=xb[:, :, :],
                    in_=xt[4:, :, c0 : c0 + tb].rearrange("j p c -> p j c"),
                )

                def xk(k):
                    return xa[:, k, :] if k < 4 else xb[:, k - 4, :]

                # ---- phase A: Gt[i, c] for all 7168 inter rows ----
                gts = []      # fp16 tiles for chunks 0..IK16-1
                gt8s = []     # fp8 pair tiles [128, 2, tb] for the NB8 tail
                gt8_cur = None
                for u in range(IK):
                    w1sb = w1p.tile([128, HJ, 128], F16, tag="w1sb")
                    nc.sync.dma_start(
                        out=w1sb[:, :, :], in_=w1[u].rearrange("j p i -> p j i")
                    )
                    w3sb = w3p.tile([128, HJ, 128], F16, tag="w3sb")
                    nc.scalar.dma_start(
                        out=w3sb[:, :, :], in_=w3[u].rearrange("j p i -> p j i")
                    )
                    if True:
                        pg1 = pga.tile([128, tb], F32, tag="pg1")
                        pg3 = pga.tile([128, tb], F32, tag="pg3")
                        for k in range(HJ):
                            nc.tensor.matmul(
                                pg1[:, :],
                                lhsT=w1sb[:, k, :],
                                rhs=xk(k),
                                start=(k == 0),
                                stop=(k == HJ - 1),
                            )
                        for k in range(HJ):
                            nc.tensor.matmul(
                                pg3[:, :],
                                lhsT=w3sb[:, k, :],
                                rhs=xk(k),
                                start=(k == 0),
                                stop=(k == HJ - 1),
                            )
                        ssb = sip.tile([128, tb], F32, tag="ssb")
                        nc.scalar.activation(
                            ssb[:, :], pg1[:, :], mybir.ActivationFunctionType.Silu
                        )
                        if u < IK16:
                            gt = gtp.tile([128, tb], F16, tag="gt")
                            nc.vector.tensor_mul(gt[:, :], pg3[:, :], ssb[:, :])
                            gts.append(gt)
                        else:
                            # fp8 tail: w3 rows were pre-scaled by SG on host,
                            # so pg3 = a3*SG and the mul writes g*SG as fp8.
                            j8 = (u - IK16) % 2
                            if j8 == 0:
                                gt8_cur = gt8p.tile([128, 2, tb], F8, tag="gt8")
                            nc.vector.tensor_mul(
                                gt8_cur[:, j8, :], pg3[:, :], ssb[:, :]
                            )
                            if j8 == 1:
                                gt8s.append(gt8_cur)

                # fp8-tail w2 pair tiles for all h: loaded now so the 6 DMAs
                # overlap the fp16 phase-B compute below
                w28sbs = []
                for up in range(NP8):
                    w28sb = w28p.tile([128, HJ, 2, 128], F8, tag="w28sb")
                    nc.sync.dma_start(
                        out=w28sb[:, :, :, :],
                        in_=w28[:, up].rearrange("h p (two i) -> p h two i", two=2),
                    )
                    w28sbs.append(w28sb)

                # ---- phase B fp8 tail first: its drain hides under the fp16 part
                for h in range(HJ):
                    po8 = po8p.tile([128, tb], F32, tag="po8")
                    for up in range(NP8):
                        nc.tensor.matmul(
                            po8[:, :],
                            lhsT=w28sbs[up][:, h, :, :],
                            rhs=gt8s[up][:, :, :],
                            start=(up == 0),
                            stop=(up == NP8 - 1),
                            perf_mode=mybir.MatmulPerfMode.DoubleRow,
                        )
                    s8 = st8.tile([128, tb], F16, tag="s8")
                    nc.vector.tensor_copy(s8[:, :], po8[:, :])
                    nc.sync.dma_start(out=yt8[h, :, c0 : c0 + tb], in_=s8[:, :])

                # ---- phase B fp16 part: Yt[h, c] over chunks 0..IK16-1 ----
                outs = []
                for g in range(n_g16):
                    u0 = g * IKG
                    nu = min(IKG, IK16 - u0)
                    w2sbs = []
                    for uu in range(nu):
                        w2sb = w2p.tile([128, H], F16, tag="w2sb")
                        nc.scalar.dma_start(out=w2sb[:, :], in_=w2[u0 + uu])
                        w2sbs.append(w2sb)
                    for h in range(HJ):
                        po = pob.tile([128, tb], F32, tag="po")
                        hs = slice(h * 128, (h + 1) * 128)
                        for uu in range(nu):
                            nc.tensor.matmul(
                                po[:, :],
                                lhsT=w2sbs[uu][:, hs],
                                rhs=gts[u0 + uu][:, :],
                                start=(uu == 0),
                                stop=(uu == nu - 1),
                            )
                        if g == 0:
                            ot = otp.tile([128, tb], F16, tag="ot")
                            nc.vector.tensor_copy(ot[:, :], po[:, :])
                            outs.append(ot)
                        else:
                            nc.vector.tensor_add(outs[h][:, :], outs[h][:, :], po[:, :])
                            if g == n_g16 - 1:
                                nc.sync.dma_start(
                                    out=yt[h, :, c0 : c0 + tb], in_=outs[h][:, :]
                                )


    legalize_single_wait(nc)
    _programs[C] = nc
    return nc


def _routing(x, gate_weight):
    """Replicate the reference router bitwise-closely: jax on CPU, same ops."""
    import jax
    import jax.numpy as jnp

    cpu = jax.devices("cpu")[0]
    with jax.default_device(cpu):
        router_logits = jnp.asarray(x) @ jnp.asarray(gate_weight).T
        probs = jax.nn.softmax(router_logits.astype(jnp.float32), axis=-1)
        top_w, top_idx = jax.lax.top_k(probs, TOPK)
        top_w = top_w / jnp.sum(top_w, axis=-1, keepdims=True)
        top_w = top_w.astype(x.dtype)
        return np.asarray(top_w), np.asarray(top_idx)


def kernel(hidden_states, gate_weight, w1_weight, w3_weight, w2_weight):
    import os

    x = np.asarray(hidden_states, dtype=np.float32)
    T = x.shape[0]
    top_w, top_idx = _routing(x, np.asarray(gate_weight, dtype=np.float32))

    tok_ids = []
    tok_w = []
    for e in range(E):
        rows, cols = np.nonzero(top_idx == e)
        tok_ids.append(rows)
        tok_w.append(top_w[rows, cols].astype(np.float32))
    C = max(512, math.ceil(max(len(t) for t in tok_ids) / 16) * 16)

    f16 = np.float16
    e4 = ml_dtypes.float8_e4m3
    in_maps = []
    for e in range(E):
        n_e = len(tok_ids[e])
        xg = np.zeros((C, H), dtype=f16)
        xg[:n_e] = x[tok_ids[e]]
        xt = np.ascontiguousarray(xg.T).reshape(HJ, 128, C)

        w1t = np.ascontiguousarray(
            np.asarray(w1_weight[e], dtype=f16).reshape(IK, 128, HJ, 128)
            .transpose(0, 2, 3, 1)
        )
        # w3 rows of the fp8 tail chunks carry the SG scale so the phase-A
        # mul emits g*SG ready for fp8 quantization.
        w3s = np.asarray(w3_weight[e], dtype=np.float32).copy()
        w3s[IK16 * 128 :] *= SG
        w3t = np.ascontiguousarray(
            w3s.astype(f16).reshape(IK, 128, HJ, 128).transpose(0, 2, 3, 1)
        )
        # fp16 phase-B weights: first IK16 chunks, [IK16, 128, H] (int-major)
        w2f = np.asarray(w2_weight[e], dtype=np.float32)  # [H, I]
        w2t = np.ascontiguousarray(
            w2f[:, : IK16 * 128].T.astype(f16)
        ).reshape(IK16, 128, H)
        # fp8 tail pairs, h-major: w28[h, up, p, (j i)] =
        #   w2[h*128 + i, (IK16 + up*2 + j)*128 + p] * SW
        w28f = np.clip(w2f[:, IK16 * 128 :] * SW, -240.0, 240.0)  # [H, NB8*128]
        w28t = np.ascontiguousarray(
            w28f.reshape(HJ, 128, NP8, 2, 128)  # [h, i, up, j, p]
            .transpose(0, 2, 4, 3, 1)           # [h, up, p, j, i]
            .reshape(HJ, NP8, 128, 256)
            .astype(e4)
        )
        in_maps.append({"xt": xt, "w1": w1t, "w3": w3t, "w2": w2t, "w28": w28t})

    nc = _build_program(C)
    trace = os.environ.get("BASS_MOE_TRACE", "") == "1"
    res = None
    if trace:
        import concourse.bass_utils as bu

        orig_upload = bu.upload_artifacts
        bu.upload_artifacts = lambda tmpdir: f"local://{tmpdir}"
        tdir = os.environ.get("BASS_MOE_TRACE_DIR") or None
        try:
            res = run_bass_kernel_spmd(
                nc, in_maps, list(range(E)), trace=True, tmpdir=tdir
            )
        except Exception as exc:
            print(f"trace path failed ({type(exc).__name__}: {exc}); rerunning untraced", flush=True)
            res = None
        finally:
            bu.upload_artifacts = orig_upload
    if res is None:
        res = run_bass_kernel_spmd(nc, in_maps, list(range(E)))
    global last_exec_time_ns, last_results
    last_exec_time_ns = res.exec_time_ns
    last_results = res

    out = np.zeros((T, H), dtype=np.float32)
    inv8 = 1.0 / (SG * SW)
    for e in range(E):
        n_e = len(tok_ids[e])
        yt = res.results[e]["yt"].astype(np.float32).reshape(H, C)
        yt8 = res.results[e]["yt8"].astype(np.float32).reshape(H, C)
        yfull = yt + yt8 * inv8
        out[tok_ids[e]] += tok_w[e][:, None] * yfull[:, :n_e].T
    return out
